# revision 52
# baseline (speedup 1.0000x reference)
"""Trainium2 Bass kernel for the masked note-accuracy loss.

Reference math (per sequence n):
    pred      = (sigmoid(x) > 0.5) = (x > 0)
    S_n       = sum_{t,d} pred * target                     (tru_pos)
    A[n,t]    = false_pos + false_neg = sum_d |pred - target|
    ratio     = S_n / (S_n + A[n,t]) = 2S_n / (2S_n + 2A[n,t])
    acc_n     = sum_{t<T_n} ratio / T_n,   T_n = sum_t mask[n,t]
    out       = sum_n acc_n

Sharding: data-parallel over N=128 sequences -> 16 per core on 8 cores;
the host sums the 8 per-core partial scalars.

Per-core pipeline, one sequence per step ([T,D] loaded as a [128,16,88]
tile, t = p*16+k, 5632B contiguous per partition; x via the SP HWDGE
queue, target via the ACT queue).  The mask is loaded ONCE in raw
contiguous layout (1KB runs, ~2.5x less fabric time than the permuted
layout); T_i and the valid mask are rebuilt on-chip from an iota-vs-T
compare -- which matches the reference's (t < T_i) semantics exactly:
  DVE pass1: V = (x>0) - target (bf16), accum_out = per-partition (P-Q)
  DVE pass2: -A[t] = negated segmented abs-reduce of V over d
  ACT pass : Copy(2*target) with accum_out = per-partition 2Q
  (separate stats tiles per writer engine -- a shared tile would
   serialize ACT behind DVE on tile-granular WAW tracking)
  mini epilogue (tiny; hidden under later sequences' DMAs):
    PE colsums -> one psum row [-A(16) | P-Q | 2Q]; its full reduce is
    directly 2S (sign trick).  PE broadcasts 2S to all partitions;
    den = -2*(-A) + 2S;  rat = recip(den) * 2S * mask;  PE colsum;
    acc += sum_t(rat) / T_n.
Final: one 4-byte DMA of the accumulated scalar.

The colsum matmuls use an all-ones [128,128] stationary so every
partition receives the column sums directly -- 2S becomes a
per-partition free-dim reduce with no PE broadcast round-trip, and PE
never reloads weights between minis.

Modeled (TimelineSim cost model) at 75.3 us/core vs the 65.0 us
HBM-stream roofline (23.2 MB/core at 360 GB/s); the gap is the Tile
lead-in/tail barriers, the last sequence's exposed DVE passes, and
the final reduction chain.
"""

import numpy as np

import concourse.bacc as bacc
import concourse.tile as tile
from concourse import mybir
from concourse.alu_op_type import AluOpType
from concourse.bass_utils import run_bass_kernel_spmd

N, T, D = 128, 2048, 88
N_CORES = 8
NS = N // N_CORES
P = 128
K = T // P

_cached_nc = None

USE_BF16_V = True
USE_NEGATE = True


def _build():
    f32 = mybir.dt.float32
    vdt = mybir.dt.bfloat16 if USE_BF16_V else f32
    nc = bacc.Bacc("TRN2", target_bir_lowering=False, debug=False,
                   num_devices=N_CORES)
    xd = nc.dram_tensor("output", [NS, T, D], f32, kind="ExternalInput")
    yd = nc.dram_tensor("target", [NS, T, D], f32, kind="ExternalInput")
    md = nc.dram_tensor("mask", [NS, T], mybir.dt.int32, kind="ExternalInput")
    od = nc.dram_tensor("partial", [1, K], f32, kind="ExternalOutput")

    AX = mybir.AxisListType.X

    with tile.TileContext(nc) as tc:
        with (
            tc.tile_pool(name="data", bufs=3) as data_pool,
            tc.tile_pool(name="work", bufs=2) as work_pool,
            tc.tile_pool(name="mini", bufs=2) as mini_pool,
            tc.tile_pool(name="singles", bufs=1) as singles,
            tc.tile_pool(name="psl", bufs=2, space="PSUM") as psum_loop,
            tc.tile_pool(name="psk", bufs=1, space="PSUM") as psum_keep,
        ):
            stA = singles.tile([P, NS, 16], f32)
            stPQ = singles.tile([P, NS], f32)
            stQ2 = singles.tile([P, NS], f32)
            maskf = singles.tile([P, NS, K], f32)
            mraw = singles.tile([P, T * NS // P], mybir.dt.int32)
            mrawf = singles.tile([P, T * NS // P], f32)
            rowsum = singles.tile([P, 1], f32)
            blockind = singles.tile([P, NS], f32)
            id16 = singles.tile([NS, NS], f32)
            t16 = singles.tile([NS, 1], f32)
            iota_t = singles.tile([P, K], mybir.dt.int32)
            ones128 = singles.tile([P, P], f32)
            inv_ti = singles.tile([1, NS], f32)
            row_ti = singles.tile([1, NS], f32)
            iota_pn = singles.tile([P, NS], mybir.dt.int32)
            iota_mn = singles.tile([NS, NS], mybir.dt.int32)
            tmp_pn = singles.tile([P, NS], f32)
            nc.vector.memset(ones128[:], 1.0)
            # blockind[p, n] = 1 iff p//8 == n, i.e. 0 <= p-8n < 8
            nc.gpsimd.iota(iota_pn[:], pattern=[[-8, NS]], base=0,
                           channel_multiplier=1)
            nc.vector.tensor_scalar(
                out=tmp_pn[:], in0=iota_pn[:], scalar1=0.0, scalar2=None,
                op0=AluOpType.is_ge)
            tmp_pn2 = singles.tile([P, NS], f32)
            nc.vector.tensor_scalar(
                out=tmp_pn2[:], in0=iota_pn[:], scalar1=8.0, scalar2=None,
                op0=AluOpType.is_lt)
            nc.vector.tensor_mul(blockind[:], tmp_pn[:], tmp_pn2[:])
            # id16[m, n] = (m == n)
            nc.gpsimd.iota(iota_mn[:], pattern=[[-1, NS]], base=0,
                           channel_multiplier=1)
            nc.vector.tensor_scalar(
                out=id16[:], in0=iota_mn[:], scalar1=0.0, scalar2=None,
                op0=AluOpType.is_equal)
            nc.gpsimd.iota(iota_t[:], pattern=[[1, K]], base=0,
                           channel_multiplier=K)

            ps_t16 = psum_keep.tile([NS, 1], f32)
            ps_ti = psum_keep.tile([1, NS], f32)
            ps_tb = psum_keep.tile([P, NS], f32)
            ps_itb = psum_keep.tile([P, NS], f32)
            ps_acc = psum_keep.tile([P, K], f32)
            sb_tb = singles.tile([P, NS], f32)
            sb_itb = singles.tile([P, NS], f32)

            def load(n):
                xt = data_pool.tile([P, K, D], f32, tag="xt")
                yt = data_pool.tile([P, K, D], f32, tag="yt")
                nc.sync.dma_start(xt[:], xd.ap()[n].rearrange("(p k) d -> p k d", p=P))
                nc.scalar.dma_start(yt[:], yd.ap()[n].rearrange("(p k) d -> p k d", p=P))
                return xt, yt

            def compute(n, xt, yt):
                v = work_pool.tile([P, K, D], vdt, tag="v")
                p1 = nc.vector.scalar_tensor_tensor(
                    out=v[:], in0=xt[:], scalar=0.0, in1=yt[:],
                    op0=AluOpType.is_gt, op1=AluOpType.subtract,
                    accum_out=stPQ[:, n : n + 1],
                )
                nc.vector.tensor_reduce(
                    out=stA[:, n, :], in_=v[:], axis=AX, op=AluOpType.add,
                    apply_absolute_value=True, negate=USE_NEGATE,
                )
                scratch = work_pool.tile([P, K, D], vdt, tag="scratch")
                nc.scalar.activation(
                    out=scratch[:], in_=yt[:],
                    func=mybir.ActivationFunctionType.Copy, scale=2.0,
                    accum_out=stQ2[:, n : n + 1],
                )
                # ---- mini epilogue ----
                # all-ones [128,128] stationary: the colsum matmul lands the
                # [-A | P-Q | 2Q] sums on EVERY partition, so 2S is just a
                # per-partition free-dim reduce -- no broadcast round-trip.
                ps_st = psum_loop.tile([P, 18], f32, tag="ps_st")
                nc.tensor.matmul(ps_st[:, 0:16], ones128[:], stA[:, n, :])
                nc.tensor.matmul(ps_st[:, 16:17], ones128[:],
                                 stPQ[:, n : n + 1])
                nc.tensor.matmul(ps_st[:, 17:18], ones128[:],
                                 stQ2[:, n : n + 1])
                s2p = mini_pool.tile([P, 1], f32, tag="s2p")
                nc.vector.tensor_reduce(
                    out=s2p[:], in_=ps_st[:], axis=AX, op=AluOpType.add)
                den = mini_pool.tile([P, K], f32, tag="den")
                nc.vector.tensor_scalar(
                    out=den[:], in0=stA[:, n, :],
                    scalar1=-2.0, scalar2=s2p[:], op0=AluOpType.mult,
                    op1=AluOpType.add)
                rec = mini_pool.tile([P, K], f32, tag="rec")
                nc.vector.reciprocal(rec[:], den[:])
                rat = mini_pool.tile([P, K], f32, tag="rat")
                nc.vector.scalar_tensor_tensor(
                    out=rat[:], in0=rec[:], scalar=s2p[:],
                    in1=maskf[:, n, :],
                    op0=AluOpType.mult, op1=AluOpType.mult)
                nc.tensor.matmul(ps_acc[:], ones128[:], rat[:],
                                 start=(n == 0), stop=(n == NS - 1))

            xt0, yt0 = load(0)
            # raw contiguous mask load (1KB runs, ~2.5x less fabric time
            # than the permuted layout); T_i and the valid mask are rebuilt
            # on-chip: valid[t] = (t < T_i), exactly the reference semantics
            nc.gpsimd.dma_start(
                mraw[:], md.ap().rearrange("n (g j) -> (n g) j", g=8))
            nc.vector.tensor_copy(mrawf[:], mraw[:])
            nc.vector.tensor_reduce(out=rowsum[:], in_=mrawf[:], axis=AX,
                                    op=AluOpType.add)
            nc.tensor.matmul(ps_t16[:], blockind[:], rowsum[:])
            nc.vector.tensor_copy(t16[:], ps_t16[:])
            nc.tensor.matmul(ps_ti[:], t16[:], id16[:])
            nc.vector.tensor_copy(row_ti[:], ps_ti[:])
            nc.vector.reciprocal(inv_ti[:], row_ti[:])
            nc.tensor.matmul(ps_tb[:], ones128[0:1, :], row_ti[:])
            nc.tensor.matmul(ps_itb[:], ones128[0:1, :], inv_ti[:])
            nc.vector.tensor_copy(sb_tb[:], ps_tb[:])
            nc.vector.tensor_copy(sb_itb[:], ps_itb[:])
            # maskf[p,n,k] = (t < T_n) / T_n : the valid mask with the
            # per-sequence 1/T_n folded in, so the ratio colsums can
            # accumulate across sequences directly in PSUM
            for n in range(NS):
                nc.vector.tensor_scalar(
                    out=maskf[:, n, :], in0=iota_t[:],
                    scalar1=sb_tb[:, n : n + 1],
                    scalar2=sb_itb[:, n : n + 1], op0=AluOpType.is_lt,
                    op1=AluOpType.mult)

            compute(0, xt0, yt0)
            for n in range(1, NS):
                xt, yt = load(n)
                compute(n, xt, yt)

            # ship the 16 accumulated column sums; the host adds them
            # (a copy is cheaper than the final reduce)
            sb_out = singles.tile([1, K], f32)
            nc.vector.tensor_copy(sb_out[:], ps_acc[0:1, :])
            nc.sync.dma_start(od.ap(), sb_out[:])

    nc.compile()
    return nc


def kernel(output, target, mask):
    global _cached_nc
    if _cached_nc is None:
        _cached_nc = _build()
    nc = _cached_nc
    output = np.asarray(output, dtype=np.float32)
    target = np.asarray(target, dtype=np.float32)
    mask = np.asarray(mask, dtype=np.int32)
    in_maps = []
    for c in range(N_CORES):
        sl = slice(c * NS, (c + 1) * NS)
        in_maps.append({
            "output": np.ascontiguousarray(output[sl]),
            "target": np.ascontiguousarray(target[sl]),
            "mask": np.ascontiguousarray(mask[sl]),
        })
    res = run_bass_kernel_spmd(nc, in_maps, list(range(N_CORES)))
    total = np.float32(0.0)
    for c in range(N_CORES):
        total = np.float32(total + np.float32(np.sum(res.results[c]["partial"], dtype=np.float64)))
    return np.float32(total)


# revision 56
# speedup vs baseline: 1.0011x; 1.0011x over previous
"""Trainium2 Bass kernel for the masked note-accuracy loss.

Reference math (per sequence n):
    pred      = (sigmoid(x) > 0.5) = (x > 0)
    S_n       = sum_{t,d} pred * target                     (tru_pos)
    A[n,t]    = false_pos + false_neg = sum_d |pred - target|
    ratio     = S_n / (S_n + A[n,t]) = 2S_n / (2S_n + 2A[n,t])
    acc_n     = sum_{t<T_n} ratio / T_n,   T_n = sum_t mask[n,t]
    out       = sum_n acc_n

Sharding: data-parallel over N=128 sequences -> 16 per core on 8 cores;
the host sums the 8 per-core partial scalars.

Per-core pipeline, one sequence per step ([T,D] loaded as a [128,16,88]
tile, t = p*16+k, 5632B contiguous per partition; x via the SP HWDGE
queue, target via the ACT queue).  The mask is loaded ONCE in raw
contiguous layout (1KB runs, ~2.5x less fabric time than the permuted
layout); T_i and the valid mask are rebuilt on-chip from an iota-vs-T
compare -- which matches the reference's (t < T_i) semantics exactly:
  DVE pass1: V = (x>0) - target (bf16), accum_out = per-partition (P-Q)
  DVE pass2: -A[t] = negated segmented abs-reduce of V over d
  ACT pass : Copy(2*target) with accum_out = per-partition 2Q
  (separate stats tiles per writer engine -- a shared tile would
   serialize ACT behind DVE on tile-granular WAW tracking)
  mini epilogue (tiny; hidden under later sequences' DMAs):
    PE colsums -> one psum row [-A(16) | P-Q | 2Q]; its full reduce is
    directly 2S (sign trick).  PE broadcasts 2S to all partitions;
    den = -2*(-A) + 2S;  rat = recip(den) * 2S * mask', where mask'
    has 1/T_n folded in; the ratio colsum matmuls ACCUMULATE across
    all sequences into one PSUM bank.
Final: one copy + 64-byte DMA of the accumulated row; host sums 16
floats per core.

The colsum matmuls use an all-ones [128,128] stationary so every
partition receives the column sums directly -- 2S becomes a
per-partition free-dim reduce with no PE broadcast round-trip, and PE
never reloads weights between minis.

Modeled (TimelineSim cost model) at 75.3 us/core vs the 65.0 us
HBM-stream roofline (23.2 MB/core at 360 GB/s); the gap is the Tile
lead-in/tail barriers, the last sequence's exposed DVE passes, and
the final reduction chain.
"""

import numpy as np

import concourse.bacc as bacc
import concourse.tile as tile
from concourse import mybir
from concourse.alu_op_type import AluOpType
from concourse.bass_utils import run_bass_kernel_spmd

N, T, D = 128, 2048, 88
N_CORES = 8
NS = N // N_CORES
P = 128
K = T // P

_cached_nc = None

USE_BF16_V = True
USE_NEGATE = True


def _build():
    f32 = mybir.dt.float32
    vdt = mybir.dt.bfloat16 if USE_BF16_V else f32
    nc = bacc.Bacc("TRN2", target_bir_lowering=False, debug=False,
                   num_devices=N_CORES)
    xd = nc.dram_tensor("output", [NS, T, D], f32, kind="ExternalInput")
    yd = nc.dram_tensor("target", [NS, T, D], f32, kind="ExternalInput")
    md = nc.dram_tensor("mask", [NS, T], mybir.dt.int32, kind="ExternalInput")
    od15 = nc.dram_tensor("partial15", [P, K], f32, kind="ExternalOutput")

    AX = mybir.AxisListType.X

    with tile.TileContext(nc) as tc:
        with (
            tc.tile_pool(name="data", bufs=3) as data_pool,
            tc.tile_pool(name="work", bufs=2) as work_pool,
            tc.tile_pool(name="mini", bufs=2) as mini_pool,
            tc.tile_pool(name="singles", bufs=1) as singles,
            tc.tile_pool(name="psl", bufs=2, space="PSUM") as psum_loop,
            tc.tile_pool(name="psk", bufs=1, space="PSUM") as psum_keep,
        ):
            stA = singles.tile([P, NS, 16], f32)
            stPQ = singles.tile([P, NS], f32)
            stQ2 = singles.tile([P, NS], f32)
            maskf = singles.tile([P, NS, K], f32)
            mraw = singles.tile([P, T * NS // P], mybir.dt.int32)
            mrawf = singles.tile([P, T * NS // P], f32)
            rowsum = singles.tile([P, 1], f32)
            blockind = singles.tile([P, NS], f32)
            id16 = singles.tile([NS, NS], f32)
            t16 = singles.tile([NS, 1], f32)
            iota_t = singles.tile([P, K], mybir.dt.int32)
            ones128 = singles.tile([P, P], f32)
            inv_ti = singles.tile([1, NS], f32)
            row_ti = singles.tile([1, NS], f32)
            iota_pn = singles.tile([P, NS], mybir.dt.int32)
            iota_mn = singles.tile([NS, NS], mybir.dt.int32)
            tmp_pn = singles.tile([P, NS], f32)
            nc.vector.memset(ones128[:], 1.0)
            # blockind[p, n] = 1 iff p//8 == n, i.e. 0 <= p-8n < 8
            nc.gpsimd.iota(iota_pn[:], pattern=[[-8, NS]], base=0,
                           channel_multiplier=1)
            nc.vector.tensor_scalar(
                out=tmp_pn[:], in0=iota_pn[:], scalar1=0.0, scalar2=None,
                op0=AluOpType.is_ge)
            tmp_pn2 = singles.tile([P, NS], f32)
            nc.vector.tensor_scalar(
                out=tmp_pn2[:], in0=iota_pn[:], scalar1=8.0, scalar2=None,
                op0=AluOpType.is_lt)
            nc.vector.tensor_mul(blockind[:], tmp_pn[:], tmp_pn2[:])
            # id16[m, n] = (m == n)
            nc.gpsimd.iota(iota_mn[:], pattern=[[-1, NS]], base=0,
                           channel_multiplier=1)
            nc.vector.tensor_scalar(
                out=id16[:], in0=iota_mn[:], scalar1=0.0, scalar2=None,
                op0=AluOpType.is_equal)
            nc.gpsimd.iota(iota_t[:], pattern=[[1, K]], base=0,
                           channel_multiplier=K)

            ps_t16 = psum_keep.tile([NS, 1], f32)
            ps_ti = psum_keep.tile([1, NS], f32)
            ps_tb = psum_keep.tile([P, NS], f32)
            ps_itb = psum_keep.tile([P, NS], f32)
            ps_acc = psum_keep.tile([P, K], f32)
            sb_tb = singles.tile([P, NS], f32)
            sb_itb = singles.tile([P, NS], f32)

            def load(n):
                xt = data_pool.tile([P, K, D], f32, tag="xt")
                yt = data_pool.tile([P, K, D], f32, tag="yt")
                nc.sync.dma_start(xt[:], xd.ap()[n].rearrange("(p k) d -> p k d", p=P))
                nc.scalar.dma_start(yt[:], yd.ap()[n].rearrange("(p k) d -> p k d", p=P))
                return xt, yt

            def compute(n, xt, yt):
                v = work_pool.tile([P, K, D], vdt, tag="v")
                p1 = nc.vector.scalar_tensor_tensor(
                    out=v[:], in0=xt[:], scalar=0.0, in1=yt[:],
                    op0=AluOpType.is_gt, op1=AluOpType.subtract,
                    accum_out=stPQ[:, n : n + 1],
                )
                nc.vector.tensor_reduce(
                    out=stA[:, n, :], in_=v[:], axis=AX, op=AluOpType.add,
                    apply_absolute_value=True, negate=USE_NEGATE,
                )
                scratch = work_pool.tile([P, K, D], vdt, tag="scratch")
                nc.scalar.activation(
                    out=scratch[:], in_=yt[:],
                    func=mybir.ActivationFunctionType.Copy, scale=2.0,
                    accum_out=stQ2[:, n : n + 1],
                )
                # ---- mini epilogue ----
                # all-ones [128,128] stationary: the colsum matmul lands the
                # [-A | P-Q | 2Q] sums on EVERY partition, so 2S is just a
                # per-partition free-dim reduce -- no broadcast round-trip.
                ps_st = psum_loop.tile([P, 18], f32, tag="ps_st")
                nc.tensor.matmul(ps_st[:, 0:16], ones128[:], stA[:, n, :])
                nc.tensor.matmul(ps_st[:, 16:17], ones128[:],
                                 stPQ[:, n : n + 1])
                nc.tensor.matmul(ps_st[:, 17:18], ones128[:],
                                 stQ2[:, n : n + 1])
                s2p = mini_pool.tile([P, 1], f32, tag="s2p")
                nc.vector.tensor_reduce(
                    out=s2p[:], in_=ps_st[:], axis=AX, op=AluOpType.add)
                den = mini_pool.tile([P, K], f32, tag="den")
                nc.vector.tensor_scalar(
                    out=den[:], in0=stA[:, n, :],
                    scalar1=-2.0, scalar2=s2p[:], op0=AluOpType.mult,
                    op1=AluOpType.add)
                rec = mini_pool.tile([P, K], f32, tag="rec")
                nc.vector.reciprocal(rec[:], den[:])
                rat = mini_pool.tile([P, K], f32, tag="rat")
                nc.vector.scalar_tensor_tensor(
                    out=rat[:], in0=rec[:], scalar=s2p[:],
                    in1=maskf[:, n, :],
                    op0=AluOpType.mult, op1=AluOpType.mult)
                if n < NS - 1:
                    nc.tensor.matmul(ps_acc[:], ones128[:], rat[:],
                                     start=(n == 0), stop=(n == NS - 2))
                else:
                    # last sequence: fold the 0..14 accumulated column sums
                    # into row 0 of its ratio tile and ship the whole tile;
                    # the host sums it (pure reduction).  Skips the colsum
                    # matmul, the PSUM copy, and a second DMA on the tail.
                    nc.vector.tensor_tensor(
                        out=rat[0:1, :], in0=rat[0:1, :],
                        in1=ps_acc[0:1, :], op=AluOpType.add)
                    nc.sync.dma_start(od15.ap(), rat[:])

            xt0, yt0 = load(0)
            # raw contiguous mask load (1KB runs, ~2.5x less fabric time
            # than the permuted layout); T_i and the valid mask are rebuilt
            # on-chip: valid[t] = (t < T_i), exactly the reference semantics
            nc.gpsimd.dma_start(
                mraw[:], md.ap().rearrange("n (g j) -> (n g) j", g=8))
            nc.vector.tensor_copy(mrawf[:], mraw[:])
            nc.vector.tensor_reduce(out=rowsum[:], in_=mrawf[:], axis=AX,
                                    op=AluOpType.add)
            nc.tensor.matmul(ps_t16[:], blockind[:], rowsum[:])
            nc.vector.tensor_copy(t16[:], ps_t16[:])
            nc.tensor.matmul(ps_ti[:], t16[:], id16[:])
            nc.vector.tensor_copy(row_ti[:], ps_ti[:])
            nc.vector.reciprocal(inv_ti[:], row_ti[:])
            nc.tensor.matmul(ps_tb[:], ones128[0:1, :], row_ti[:])
            nc.tensor.matmul(ps_itb[:], ones128[0:1, :], inv_ti[:])
            nc.vector.tensor_copy(sb_tb[:], ps_tb[:])
            nc.vector.tensor_copy(sb_itb[:], ps_itb[:])
            # maskf[p,n,k] = (t < T_n) / T_n : the valid mask with the
            # per-sequence 1/T_n folded in, so the ratio colsums can
            # accumulate across sequences directly in PSUM
            for n in range(NS):
                nc.vector.tensor_scalar(
                    out=maskf[:, n, :], in0=iota_t[:],
                    scalar1=sb_tb[:, n : n + 1],
                    scalar2=sb_itb[:, n : n + 1], op0=AluOpType.is_lt,
                    op1=AluOpType.mult)

            compute(0, xt0, yt0)
            for n in range(1, NS):
                xt, yt = load(n)
                compute(n, xt, yt)


    nc.compile()
    return nc


def kernel(output, target, mask):
    global _cached_nc
    if _cached_nc is None:
        _cached_nc = _build()
    nc = _cached_nc
    output = np.asarray(output, dtype=np.float32)
    target = np.asarray(target, dtype=np.float32)
    mask = np.asarray(mask, dtype=np.int32)
    in_maps = []
    for c in range(N_CORES):
        sl = slice(c * NS, (c + 1) * NS)
        in_maps.append({
            "output": np.ascontiguousarray(output[sl]),
            "target": np.ascontiguousarray(target[sl]),
            "mask": np.ascontiguousarray(mask[sl]),
        })
    res = run_bass_kernel_spmd(nc, in_maps, list(range(N_CORES)))
    total = np.float32(0.0)
    for c in range(N_CORES):
        part = np.sum(res.results[c]["partial15"], dtype=np.float64)
        total = np.float32(total + np.float32(part))
    return np.float32(total)
